# revision 48
# baseline (speedup 1.0000x reference)
"""Trainium2 Bass kernel: multi-head attention (dense transformer block).

Computation (per batch b):
    Q = x @ Wq + bq ; K = x @ Wk + bk ; V = x @ Wv + bv        (per head)
    P = exp((Q @ K^T) / sqrt(Dh))                   (no max-subtraction needed:
                                                     scores are O(1) by construction)
    out = sum_h (P @ V / rowsum(P)) @ Wd[h] + bd

Sharding (data + tensor parallel): 8 cores; core c handles batch b = c // 4
and the 4 heads starting at 4*(c % 4). Each core computes a partial [L, D]
output; the host sums the 4 partials per batch and adds bd.

Host-side layout prep: x is pre-transposed to x^T and pre-cast to bf16 (the
same rounding the kernel used to do on-chip), weights are pre-packed into
their exact on-chip bf16 layouts. All DMAs are contiguous HWDGE transfers,
and the on-chip x-transpose/cast pipeline (which dominated the old lead-in)
disappears entirely.

Schedule: hand-interleaved emission so the PE never idles and the Act engine
runs exp (its ~128us is the #2 engine floor) nearly continuously:
  - K/Q chunks follow each x^T l-chunk DMA; scores stream right behind.
  - pair-1 QKV, V-proj and the ec0 out-projection are emitted inside other
    blocks' exp-lag windows as PE filler.
  - Act engine does ONLY exp; drains/biases live on DVE & Pool.
  - V computed as V^T (weight-stationary J=512, LDWEIGHTS hidden) then
    PE-transposed back; ones-columns give the softmax denominator for free.
  - normalize: reciprocal_approx_fast (DVE) + Pool multiply.
  - out-proj per ec with per-tile y DMA, overlapped with ec1 compute.
"""

import os
import sys
from contextlib import ExitStack

import numpy as np
import ml_dtypes

for _p in ("/opt/trn_rl_repo", "/root/.axon_site/_ro/trn_rl_repo"):
    if os.path.isdir(_p) and _p not in sys.path:
        sys.path.append(_p)

import concourse.bass as bass
import concourse.tile as tile
from concourse import bacc, mybir
from concourse.bass import ds, ts
from concourse.bass_utils import run_bass_kernel_spmd
from concourse.masks import make_identity

F32 = mybir.dt.float32
BF16 = mybir.dt.bfloat16
BF16_NP = ml_dtypes.bfloat16

# Problem sizes (hardcoded per contract).
DMODEL, HEADS, DHEAD = 1024, 16, 64
B, L = 2, 2048
NCORES = 8
H_PER_CORE = B * HEADS // NCORES          # 4 heads per core
NPAIR = H_PER_CORE // 2                   # head pairs per core (= 2)
P = 128                                   # partitions
KT = DMODEL // P                          # 8 k-tiles over dmodel
NLT = L // P                              # 16 l-tiles
LCH = 512                                 # matmul free-dim chunk (one psum bank)
NLC = L // LCH                            # 4 l-chunks
ECH = 1024                                # exp chunk (2 psum banks)
NEC = L // ECH                            # 2 exp chunks
MCH = 512                                 # m-chunk for out-proj
NMC = DMODEL // MCH


def build_nc():
    """Build the SPMD Bass program for one core."""
    nc = bacc.Bacc("TRN2", target_bir_lowering=False, debug=False,
                   num_devices=NCORES)

    # Inputs are pre-packed host-side into on-chip layouts (see shard_inputs).
    xt_d = nc.dram_tensor("xt", [P, NLC, KT, LCH], BF16, kind="ExternalInput").ap()
    w_d = nc.dram_tensor("w", [P, NPAIR, 3, KT, P], BF16, kind="ExternalInput").ap()
    wd_d = nc.dram_tensor("wd", [P, NPAIR, DMODEL], BF16, kind="ExternalInput").ap()
    bias_d = nc.dram_tensor("bias", [P, 3, NPAIR], F32, kind="ExternalInput").ap()
    bvr_d = nc.dram_tensor("bvr", [P, NPAIR * P], F32, kind="ExternalInput").ap()
    y_d = nc.dram_tensor("y", [L, DMODEL], F32, kind="ExternalOutput").ap()

    with ExitStack() as ctx:
        tc = ctx.enter_context(tile.TileContext(nc))
        _body(nc, tc, ctx, xt_d, w_d, wd_d, bias_d, bvr_d, y_d)
    nc.compile()
    return nc


def _body(nc, tc, ctx, xt_d, w_d, wd_d, bias_d, bvr_d, y_d):
    const = ctx.enter_context(tc.tile_pool(name="const", bufs=1))
    sb = ctx.enter_context(tc.tile_pool(name="sb", bufs=1))
    psum = ctx.enter_context(tc.tile_pool(name="psum", bufs=1, space="PSUM"))

    ident = const.tile([P, P], BF16)
    make_identity(nc, ident)

    # ---- persistent SBUF tensors (DMA'd directly, no staging) ----
    xt = sb.tile([P, NLC, KT, LCH], BF16)                # x^T, bf16
    w_sb = const.tile([P, NPAIR, 3, KT, P], BF16)        # Wq/Wk/Wv per pair
    wd_sb = const.tile([P, NPAIR, DMODEL], BF16)
    bias_sb = const.tile([P, 3, NPAIR], F32)
    bv_rep = const.tile([P, NPAIR * P], F32)
    kT = [sb.tile([P, L], BF16, tag="kT", bufs=NPAIR, name=f"kT{p}")
          for p in range(NPAIR)]
    qT = [sb.tile([P, L], BF16, tag="qT", bufs=NPAIR, name=f"qT{p}")
          for p in range(NPAIR)]
    vt = [sb.tile([P, NLT, 2 * P], BF16, tag="vt", bufs=NPAIR, name=f"vt{p}")
          for p in range(NPAIR)]
    o_norm = sb.tile([P, NPAIR, L], BF16)

    # ---- emission helpers (order of calls == engine queue order) ----

    def emit_kq_chunk(p, i, lc):
        """One 512-wide chunk of K^T (i=1) or Q^T (i=0) for pair p."""
        dst = qT[p] if i == 0 else kT[p]
        ps = psum.tile([P, LCH], F32, tag="qkvp", bufs=2, name="qkvps")
        for kt in range(KT):
            nc.tensor.matmul(ps, lhsT=w_sb[:, p, i, kt],
                             rhs=xt[:, lc, kt],
                             start=(kt == 0), stop=(kt == KT - 1))
        nc.vector.tensor_scalar_add(dst[:, ds(lc * LCH, LCH)], ps,
                                    bias_sb[:, i, p:p + 1])

    def emit_v_ones(p):
        """memset the ones-columns of vt[p] (softmax denominator trick)."""
        v4 = vt[p].rearrange("q l (b c) -> q l b c", b=2)
        nc.gpsimd.memset(v4[:, :, :, DHEAD:P], 1.0)

    def emit_v_chunk(p, lc):
        """V for l-tiles 4lc..4lc+3 of pair p: V^T with weight-stationary
        J=512 matmuls (LDWEIGHTS hidden), PE-transpose back to [l', d],
        bias added on the strided drain."""
        vps = psum.tile([P, LCH], F32, tag="qkvp", bufs=2, name="vps")
        for kt in range(KT):
            nc.tensor.matmul(vps, lhsT=w_sb[:, p, 2, kt],
                             rhs=xt[:, lc, kt],
                             start=(kt == 0), stop=(kt == KT - 1))
        vts = sb.tile([P, LCH], BF16, tag="vts", bufs=1)
        nc.vector.tensor_copy(vts, vps)
        tp = psum.tile([P, 4, P], BF16, tag="op", bufs=2, name="vtp")
        for j in range(4):
            nc.tensor.transpose(tp[:, j], vts[:, ds(j * P, P)], ident)
        bvs = bv_rep[:, ds(p * P, P)].rearrange("q (b c) -> q b c", b=2)
        for j in range(4):
            lt = 4 * lc + j
            dst = vt[p].rearrange("q l (b c) -> q l b c", b=2)[:, lt, :, 0:DHEAD]
            nc.vector.tensor_add(dst, tp[:, j].rearrange("q (b c) -> q b c", b=2), bvs)

    def emit_scores(p, ec, lt, pt_tiles):
        """Scores + exp for one key l-tile of (pair, ec): 2 heads dual-tile."""
        for h in range(2):
            sp = psum.tile([P, ECH], F32, tag="sc", bufs=2, name="sp")
            for sub in range(ECH // LCH):
                nc.tensor.matmul(
                    sp[:, ds(sub * LCH, LCH)],
                    lhsT=kT[p][ds(64 * h, 64), ds(lt * P, P)],
                    rhs=qT[p][ds(64 * h, 64), ds(ec * ECH + sub * LCH, LCH)],
                    start=True, stop=True)
            pt = sb.tile([P, ECH], BF16, tag="pt", bufs=40)
            nc.scalar.activation(pt, sp, func=mybir.ActivationFunctionType.Exp,
                                 scale=1.0 / np.sqrt(DHEAD))
            pt_tiles[h][lt] = pt

    def emit_attend_sub(p, ec, sub, pt_tiles, filler=()):
        """One 512-query sub-chunk: h0+h1 chains interleaved so both track
        the exp stream; then normalize both heads. `filler` thunks are
        drizzled between chain steps (PE slack)."""
        filler = list(filler)
        lc = ec * ECH + sub * LCH
        ops = [psum.tile([P, LCH], F32, tag="op", bufs=2, name=f"oph{h}")
               for h in range(2)]
        for lt in range(NLT):
            for h in range(2):
                nc.tensor.matmul(
                    ops[h], lhsT=vt[p][:, lt, ds(P * h, P)],
                    rhs=pt_tiles[h][lt][:, ds(sub * LCH, LCH)],
                    start=(lt == 0), stop=(lt == NLT - 1))
            if lt % 2 == 1 and filler:
                filler.pop(0)()
        for h in range(2):
            op = ops[h]
            # rows 64..127 hold the softmax denominator (ones columns).
            # Builtin copies handle the partition shift 64->0; the custom
            # approx-reciprocal then runs partition-aligned.
            os_sb = sb.tile([DHEAD, LCH], F32, tag="os", bufs=2)
            nc.vector.tensor_copy(os_sb, op[0:DHEAD, :])
            den = sb.tile([DHEAD, LCH], F32, tag="den", bufs=2)
            nc.vector.tensor_copy(den, op[DHEAD:P, :])
            rs = sb.tile([DHEAD, LCH], F32, tag="rs", bufs=2)
            nc.vector.reciprocal_approx_fast(rs, den)
            nc.gpsimd.tensor_mul(
                o_norm[ds(64 * h, 64), p, ds(lc, LCH)], os_sb, rs)

    def emit_attend(p, ec, pt_tiles):
        for sub in range(ECH // LCH):
            emit_attend_sub(p, ec, sub, pt_tiles)

    _opj = [0]

    def emit_outproj_unit(lt, mc, tail=False):
        """One [128, 512] Y tile. In the tail, rotate psum tags (all free
        by then) to deepen the pipeline, split drains Act/DVE, and fan the
        y DMAs across three queues."""
        k = _opj[0]
        _opj[0] += 1
        tag = "qkvp"
        yp = psum.tile([P, MCH], F32, tag=tag, bufs=2, name=f"yp_{tag}")
        for p in range(NPAIR):
            nc.tensor.matmul(
                yp, lhsT=o_norm[:, p, ds(lt * P, P)],
                rhs=wd_sb[:, p, ds(mc * MCH, MCH)],
                start=(p == 0), stop=(p == NPAIR - 1))
        ys = sb.tile([P, MCH], F32, tag="ys", bufs=3)
        if tail:
            nc.scalar.copy(ys, yp)
        else:
            nc.vector.tensor_copy(ys, yp)
        dst = y_d[ds(lt * P, P), ds(mc * MCH, MCH)]
        if tail and k % 3 == 1:
            nc.scalar.dma_start(dst, ys)
        elif tail and k % 3 == 2:
            nc.gpsimd.dma_start(dst, ys)
        else:
            nc.sync.dma_start(dst, ys)

    def emit_outproj_g(lts, tail=False):
        for lt in lts:
            for mc in range(NMC):
                emit_outproj_unit(lt, mc, tail)

    # ================= global emission order =================
    pt_a = [[None] * NLT, [None] * NLT]   # pt tiles for the active block
    p0, p1 = 0, 1

    # Input DMAs: K/Q weights of pair 0 first (they gate the first chunks),
    # then x^T chunks; everything else rides the gpsimd queue in parallel.
    nc.scalar.dma_start(w_sb[:, p0, 1], w_d[:, p0, 1])
    nc.scalar.dma_start(w_sb[:, p0, 0], w_d[:, p0, 0])
    nc.gpsimd.dma_start(bias_sb, bias_d)
    nc.gpsimd.dma_start(w_sb[:, p0, 2], w_d[:, p0, 2])
    nc.gpsimd.dma_start(bv_rep, bvr_d)
    for kq in range(4):                               # chunk 0 in quarters
        nc.sync.dma_start(xt[:, 0, ds(2 * kq, 2)], xt_d[:, 0, ds(2 * kq, 2)])
    for lc in range(1, NLC):
        nc.sync.dma_start(xt[:, lc, 0:KT // 2], xt_d[:, lc, 0:KT // 2])
        nc.sync.dma_start(xt[:, lc, KT // 2:], xt_d[:, lc, KT // 2:])
    nc.gpsimd.dma_start(w_sb[:, p1], w_d[:, p1])
    nc.gpsimd.dma_start(wd_sb, wd_d)

    # K/Q(ec0) p0 + scores(p0, ec0), streaming behind the x^T chunk DMAs.
    # scores(lt) needs K chunk lt//4 and BOTH Q chunks of ec0.
    for g in range(4):
        emit_kq_chunk(p0, 1, g)                       # K chunk g
        if g == 0:
            emit_kq_chunk(p0, 0, 0)                   # Q ec0 chunk 0
        elif g == 1:
            emit_kq_chunk(p0, 0, 1)                   # Q ec0 chunk 1
            for lt in range(0, 8):
                emit_scores(p0, 0, lt, pt_a)
        else:
            for lt in range(4 * g, 4 * g + 4):
                emit_scores(p0, 0, lt, pt_a)

    emit_v_ones(p0)
    for lc in range(NLC):                             # V p0 (dep of attend)
        emit_v_chunk(p0, lc)
    emit_kq_chunk(p0, 0, 2)                           # Q p0 ec1 [filler]
    emit_kq_chunk(p0, 0, 3)
    pt_b = [[None] * NLT, [None] * NLT]
    for lt in range(0, 4):                            # boundary prefill
        emit_scores(p0, 1, lt, pt_b)
    emit_attend(p0, 0, pt_a)

    for lt in range(4, NLT):
        emit_scores(p0, 1, lt, pt_b)
    for lc in range(NLC):                             # K p1 [filler]
        emit_kq_chunk(p1, 1, lc)
    emit_kq_chunk(p1, 0, 0)                           # Q p1 ec0
    emit_kq_chunk(p1, 0, 1)
    pt_a = [[None] * NLT, [None] * NLT]
    for lt in range(0, 4):                            # boundary prefill
        emit_scores(p1, 0, lt, pt_a)
    emit_attend(p0, 1, pt_b)

    for lt in range(4, NLT):
        emit_scores(p1, 0, lt, pt_a)
    emit_v_ones(p1)
    for lc in range(NLC):                             # V p1 [filler + dep]
        emit_v_chunk(p1, lc)
    emit_kq_chunk(p1, 0, 2)                           # Q p1 ec1
    emit_kq_chunk(p1, 0, 3)
    pt_b = [[None] * NLT, [None] * NLT]
    for lt in range(0, 4):                            # boundary prefill
        emit_scores(p1, 1, lt, pt_b)
    emit_attend_sub(p1, 0, 0, pt_a)
    emit_attend_sub(p1, 0, 1, pt_a)

    # block p1ec1: out-proj ec0 groups ride the PE slack of the Act-limited
    # scores stream; tail out-proj follows the norms immediately.
    for lt in range(4, 7):
        emit_scores(p1, 1, lt, pt_b)
    emit_outproj_g(range(0, 2))
    for lt in range(7, 10):
        emit_scores(p1, 1, lt, pt_b)
    emit_outproj_g(range(2, 4))
    for lt in range(10, 13):
        emit_scores(p1, 1, lt, pt_b)
    emit_outproj_g(range(4, 6))
    for lt in range(13, 16):
        emit_scores(p1, 1, lt, pt_b)
    emit_outproj_g(range(6, 8))
    emit_attend_sub(p1, 1, 0, pt_b)
    emit_attend_sub(p1, 1, 1, pt_b)
    emit_outproj_g(range(8, 12), tail=True)
    emit_outproj_g(range(12, 16), tail=True)          # tail


_NC_CACHE = {}


def _get_nc():
    if "nc" not in _NC_CACHE:
        _NC_CACHE["nc"] = build_nc()
    return _NC_CACHE["nc"]


def shard_inputs(x, Wq, bq, Wk, bk, Wv, bv, Wd, bd):
    """Build the 8 per-core input maps, pre-packed into on-chip layouts."""
    x = np.asarray(x, np.float32)
    Wq = np.asarray(Wq, np.float32)
    Wk = np.asarray(Wk, np.float32)
    Wv = np.asarray(Wv, np.float32)
    Wd = np.asarray(Wd, np.float32)
    bq = np.asarray(bq, np.float32)
    bk = np.asarray(bk, np.float32)
    bv = np.asarray(bv, np.float32)

    # x^T per batch in [p, lc, kt, l'] bf16 (the kernel's SBUF layout).
    xts = []
    for b in range(B):
        xT = x[b].T.astype(BF16_NP)                      # [DMODEL, L]
        xts.append(np.ascontiguousarray(
            xT.reshape(KT, P, NLC, LCH).transpose(1, 2, 0, 3)))

    in_maps = []
    for c in range(NCORES):
        b = c // (NCORES // B)
        h0 = (c % (NCORES // B)) * H_PER_CORE
        hs = slice(h0, h0 + H_PER_CORE)
        # [k, pair, i, kt, hd] bf16
        w3 = np.stack(
            [W[:, hs, :].reshape(DMODEL, NPAIR * P)
              .reshape(KT, P, NPAIR, P).transpose(1, 2, 0, 3)
             for W in (Wq, Wk, Wv)], axis=2).astype(BF16_NP)
        # [hd, pair, m] bf16
        wd = (Wd[hs].reshape(NPAIR, P, DMODEL).transpose(1, 0, 2)
              .astype(BF16_NP))
        # [hd, i, pair] f32
        b3 = np.stack([bq[hs].reshape(-1), bk[hs].reshape(-1),
                       bv[hs].reshape(-1)], axis=1).reshape(NPAIR, P, 3)
        b3 = np.ascontiguousarray(b3.transpose(1, 2, 0))
        # [part, pair*P] f32 (bv broadcast across partitions)
        bvr = np.broadcast_to(bv[hs].reshape(1, NPAIR * P), (P, NPAIR * P))
        in_maps.append({
            "xt": xts[b],
            "w": np.ascontiguousarray(w3),
            "wd": np.ascontiguousarray(wd),
            "bias": np.ascontiguousarray(b3),
            "bvr": np.ascontiguousarray(bvr, dtype=np.float32),
        })
    return in_maps


def gather_outputs(results, bd):
    """Sum partial outputs per batch and add bd."""
    out = np.zeros((B, L, DMODEL), np.float32)
    per_b = NCORES // B
    for c, res in enumerate(results):
        out[c // per_b] += res["y"]
    out += np.asarray(bd, np.float32)[None, None, :]
    return out


def kernel(x, Wq, bq, Wk, bk, Wv, bv, Wd, bd, _trace=False):
    nc = _get_nc()
    in_maps = shard_inputs(x, Wq, bq, Wk, bk, Wv, bv, Wd, bd)
    res = run_bass_kernel_spmd(nc, in_maps, list(range(NCORES)), trace=_trace)
    out = gather_outputs(res.results, bd)
    if _trace:
        kernel.last_results = res
    return out


# revision 49
# speedup vs baseline: 1.1789x; 1.1789x over previous
"""Trainium2 Bass kernel: multi-head attention (dense transformer block).

Computation (per batch b):
    Q = x @ Wq + bq ; K = x @ Wk + bk ; V = x @ Wv + bv        (per head)
    P = exp((Q @ K^T) / sqrt(Dh))                   (no max-subtraction needed:
                                                     scores are O(1) by construction)
    out = sum_h (P @ V / rowsum(P)) @ Wd[h] + bd

Sharding (data + tensor parallel): 8 cores; core c handles batch b = c // 4
and the 4 heads starting at 4*(c % 4). Each core computes a partial [L, D]
output; the host sums the 4 partials per batch and adds bd.

Host-side layout prep: x is pre-transposed to x^T and pre-cast to bf16 (the
same rounding the kernel used to do on-chip), weights are pre-packed into
their exact on-chip bf16 layouts. All DMAs are contiguous HWDGE transfers,
and the on-chip x-transpose/cast pipeline (which dominated the old lead-in)
disappears entirely.

Schedule: hand-interleaved emission so the PE never idles and the Act engine
runs exp (its ~128us is the #2 engine floor) nearly continuously:
  - K/Q chunks follow each x^T l-chunk DMA; scores stream right behind.
  - pair-1 QKV, V-proj and the ec0 out-projection are emitted inside other
    blocks' exp-lag windows as PE filler.
  - Act engine does ONLY exp; drains/biases live on DVE & Pool.
  - V computed as V^T (weight-stationary J=512, LDWEIGHTS hidden) then
    PE-transposed back; ones-columns give the softmax denominator for free.
  - normalize: reciprocal_approx_fast (DVE) + Pool multiply.
  - out-proj per ec with per-tile y DMA, overlapped with ec1 compute.
"""

import os
import sys
from contextlib import ExitStack

import numpy as np
import ml_dtypes

for _p in ("/opt/trn_rl_repo", "/root/.axon_site/_ro/trn_rl_repo"):
    if os.path.isdir(_p) and _p not in sys.path:
        sys.path.append(_p)

import concourse.bass as bass
import concourse.tile as tile
from concourse import bacc, mybir
from concourse.bass import ds, ts
from concourse.bass_utils import run_bass_kernel_spmd
from concourse.masks import make_identity

F32 = mybir.dt.float32
BF16 = mybir.dt.bfloat16
BF16_NP = ml_dtypes.bfloat16

# Problem sizes (hardcoded per contract).
DMODEL, HEADS, DHEAD = 1024, 16, 64
B, L = 2, 2048
NCORES = 8
H_PER_CORE = B * HEADS // NCORES          # 4 heads per core
NPAIR = H_PER_CORE // 2                   # head pairs per core (= 2)
P = 128                                   # partitions
KT = DMODEL // P                          # 8 k-tiles over dmodel
NLT = L // P                              # 16 l-tiles
LCH = 512                                 # matmul free-dim chunk (one psum bank)
NLC = L // LCH                            # 4 l-chunks
ECH = 1024                                # exp chunk (2 psum banks)
NEC = L // ECH                            # 2 exp chunks
MCH = 512                                 # m-chunk for out-proj
NMC = DMODEL // MCH


def build_nc():
    """Build the SPMD Bass program for one core."""
    nc = bacc.Bacc("TRN2", target_bir_lowering=False, debug=False,
                   num_devices=NCORES)

    # Inputs are pre-packed host-side into on-chip layouts (see shard_inputs).
    xt_d = nc.dram_tensor("xt", [P, NLC, KT, LCH], BF16, kind="ExternalInput").ap()
    w_d = nc.dram_tensor("w", [P, NPAIR, 3, KT, P], BF16, kind="ExternalInput").ap()
    wd_d = nc.dram_tensor("wd", [P, NPAIR, DMODEL], BF16, kind="ExternalInput").ap()
    bias_d = nc.dram_tensor("bias", [P, 3, NPAIR], F32, kind="ExternalInput").ap()
    bvr_d = nc.dram_tensor("bvr", [P, NPAIR * P], F32, kind="ExternalInput").ap()
    y_d = nc.dram_tensor("y", [L, DMODEL], BF16, kind="ExternalOutput").ap()

    with ExitStack() as ctx:
        tc = ctx.enter_context(tile.TileContext(nc))
        _body(nc, tc, ctx, xt_d, w_d, wd_d, bias_d, bvr_d, y_d)
    nc.compile()
    return nc


def _body(nc, tc, ctx, xt_d, w_d, wd_d, bias_d, bvr_d, y_d):
    const = ctx.enter_context(tc.tile_pool(name="const", bufs=1))
    sb = ctx.enter_context(tc.tile_pool(name="sb", bufs=1))
    psum = ctx.enter_context(tc.tile_pool(name="psum", bufs=1, space="PSUM"))

    ident = const.tile([P, P], BF16)
    make_identity(nc, ident)

    # ---- persistent SBUF tensors (DMA'd directly, no staging) ----
    xt = sb.tile([P, NLC, KT, LCH], BF16)                # x^T, bf16
    w_sb = const.tile([P, NPAIR, 3, KT, P], BF16)        # Wq/Wk/Wv per pair
    wd_sb = const.tile([P, NPAIR, DMODEL], BF16)
    bias_sb = const.tile([P, 3, NPAIR], F32)
    bv_rep = const.tile([P, NPAIR * P], F32)
    kT = [sb.tile([P, L], BF16, tag="kT", bufs=NPAIR, name=f"kT{p}")
          for p in range(NPAIR)]
    qT = [sb.tile([P, L], BF16, tag="qT", bufs=NPAIR, name=f"qT{p}")
          for p in range(NPAIR)]
    vt = [sb.tile([P, NLT, 2 * P], BF16, tag="vt", bufs=NPAIR, name=f"vt{p}")
          for p in range(NPAIR)]
    o_norm = sb.tile([P, NPAIR, L], BF16)

    # ---- emission helpers (order of calls == engine queue order) ----

    def emit_kq_chunk(p, i, lc):
        """One 512-wide chunk of K^T (i=1) or Q^T (i=0) for pair p."""
        dst = qT[p] if i == 0 else kT[p]
        ps = psum.tile([P, LCH], F32, tag="qkvp", bufs=2, name="qkvps")
        for kt in range(KT):
            nc.tensor.matmul(ps, lhsT=w_sb[:, p, i, kt],
                             rhs=xt[:, lc, kt],
                             start=(kt == 0), stop=(kt == KT - 1))
        nc.vector.tensor_scalar_add(dst[:, ds(lc * LCH, LCH)], ps,
                                    bias_sb[:, i, p:p + 1])

    def emit_v_ones(p):
        """memset the ones-columns of vt[p] (softmax denominator trick)."""
        v4 = vt[p].rearrange("q l (b c) -> q l b c", b=2)
        nc.gpsimd.memset(v4[:, :, :, DHEAD:P], 1.0)

    def emit_v_chunk(p, lc):
        """V for l-tiles 4lc..4lc+3 of pair p: V^T with weight-stationary
        J=512 matmuls (LDWEIGHTS hidden), PE-transpose back to [l', d],
        bias added on the strided drain."""
        vps = psum.tile([P, LCH], F32, tag="qkvp", bufs=2, name="vps")
        for kt in range(KT):
            nc.tensor.matmul(vps, lhsT=w_sb[:, p, 2, kt],
                             rhs=xt[:, lc, kt],
                             start=(kt == 0), stop=(kt == KT - 1))
        vts = sb.tile([P, LCH], BF16, tag="vts", bufs=1)
        nc.vector.tensor_copy(vts, vps)
        tp = psum.tile([P, 4, P], BF16, tag="op", bufs=2, name="vtp")
        for j in range(4):
            nc.tensor.transpose(tp[:, j], vts[:, ds(j * P, P)], ident)
        bvs = bv_rep[:, ds(p * P, P)].rearrange("q (b c) -> q b c", b=2)
        for j in range(4):
            lt = 4 * lc + j
            dst = vt[p].rearrange("q l (b c) -> q l b c", b=2)[:, lt, :, 0:DHEAD]
            nc.vector.tensor_add(dst, tp[:, j].rearrange("q (b c) -> q b c", b=2), bvs)

    def emit_scores(p, ec, lt, pt_tiles):
        """Scores + exp for one key l-tile of (pair, ec): 2 heads dual-tile."""
        for h in range(2):
            sp = psum.tile([P, ECH], F32, tag="sc", bufs=2, name="sp")
            for sub in range(ECH // LCH):
                nc.tensor.matmul(
                    sp[:, ds(sub * LCH, LCH)],
                    lhsT=kT[p][ds(64 * h, 64), ds(lt * P, P)],
                    rhs=qT[p][ds(64 * h, 64), ds(ec * ECH + sub * LCH, LCH)],
                    start=True, stop=True)
            pt = sb.tile([P, ECH], BF16, tag="pt", bufs=40)
            nc.scalar.activation(pt, sp, func=mybir.ActivationFunctionType.Exp,
                                 scale=1.0 / np.sqrt(DHEAD))
            pt_tiles[h][lt] = pt

    def emit_attend_sub(p, ec, sub, pt_tiles, filler=()):
        """One 512-query sub-chunk: h0+h1 chains interleaved so both track
        the exp stream; then normalize both heads. `filler` thunks are
        drizzled between chain steps (PE slack)."""
        filler = list(filler)
        lc = ec * ECH + sub * LCH
        ops = [psum.tile([P, LCH], F32, tag="op", bufs=2, name=f"oph{h}")
               for h in range(2)]
        for lt in range(NLT):
            for h in range(2):
                nc.tensor.matmul(
                    ops[h], lhsT=vt[p][:, lt, ds(P * h, P)],
                    rhs=pt_tiles[h][lt][:, ds(sub * LCH, LCH)],
                    start=(lt == 0), stop=(lt == NLT - 1))
            if lt % 2 == 1 and filler:
                filler.pop(0)()
        for h in range(2):
            op = ops[h]
            # rows 64..127 hold the softmax denominator (ones columns).
            # Builtin copies handle the partition shift 64->0; the custom
            # approx-reciprocal then runs partition-aligned.
            os_sb = sb.tile([DHEAD, LCH], F32, tag="os", bufs=2)
            nc.vector.tensor_copy(os_sb, op[0:DHEAD, :])
            den = sb.tile([DHEAD, LCH], F32, tag="den", bufs=2)
            nc.vector.tensor_copy(den, op[DHEAD:P, :])
            rs = sb.tile([DHEAD, LCH], F32, tag="rs", bufs=2)
            nc.vector.reciprocal_approx_fast(rs, den)
            nc.gpsimd.tensor_mul(
                o_norm[ds(64 * h, 64), p, ds(lc, LCH)], os_sb, rs)

    def emit_attend(p, ec, pt_tiles):
        for sub in range(ECH // LCH):
            emit_attend_sub(p, ec, sub, pt_tiles)

    _opj = [0]

    def emit_outproj_unit(lt, mc, tail=False):
        """One [128, 512] Y tile. In the tail, rotate psum tags (all free
        by then) to deepen the pipeline, split drains Act/DVE, and fan the
        y DMAs across three queues."""
        k = _opj[0]
        _opj[0] += 1
        tag = "qkvp"
        yp = psum.tile([P, MCH], F32, tag=tag, bufs=2, name=f"yp_{tag}")
        for p in range(NPAIR):
            nc.tensor.matmul(
                yp, lhsT=o_norm[:, p, ds(lt * P, P)],
                rhs=wd_sb[:, p, ds(mc * MCH, MCH)],
                start=(p == 0), stop=(p == NPAIR - 1))
        ys = sb.tile([P, MCH], BF16, tag="ys", bufs=3)
        if tail:
            nc.scalar.copy(ys, yp)
        else:
            nc.vector.tensor_copy(ys, yp)
        dst = y_d[ds(lt * P, P), ds(mc * MCH, MCH)]
        if tail and k % 3 == 1:
            nc.scalar.dma_start(dst, ys)
        elif tail and k % 3 == 2:
            nc.gpsimd.dma_start(dst, ys)
        else:
            nc.sync.dma_start(dst, ys)

    def emit_outproj_g(lts, tail=False):
        for lt in lts:
            for mc in range(NMC):
                emit_outproj_unit(lt, mc, tail)

    # ================= global emission order =================
    pt_a = [[None] * NLT, [None] * NLT]   # pt tiles for the active block
    p0, p1 = 0, 1

    # Input DMAs: K/Q weights of pair 0 first (they gate the first chunks),
    # then x^T chunks; everything else rides the gpsimd queue in parallel.
    nc.scalar.dma_start(w_sb[:, p0, 1], w_d[:, p0, 1])
    nc.scalar.dma_start(w_sb[:, p0, 0], w_d[:, p0, 0])
    nc.gpsimd.dma_start(bias_sb, bias_d)
    nc.gpsimd.dma_start(w_sb[:, p0, 2], w_d[:, p0, 2])
    nc.gpsimd.dma_start(bv_rep, bvr_d)
    for kq in range(4):                               # chunk 0 in quarters
        nc.sync.dma_start(xt[:, 0, ds(2 * kq, 2)], xt_d[:, 0, ds(2 * kq, 2)])
    for lc in range(1, NLC):
        nc.sync.dma_start(xt[:, lc, 0:KT // 2], xt_d[:, lc, 0:KT // 2])
        nc.sync.dma_start(xt[:, lc, KT // 2:], xt_d[:, lc, KT // 2:])
    nc.gpsimd.dma_start(w_sb[:, p1], w_d[:, p1])
    nc.gpsimd.dma_start(wd_sb, wd_d)

    # K/Q(ec0) p0 + scores(p0, ec0), streaming behind the x^T chunk DMAs.
    # scores(lt) needs K chunk lt//4 and BOTH Q chunks of ec0.
    for g in range(4):
        emit_kq_chunk(p0, 1, g)                       # K chunk g
        if g == 0:
            emit_kq_chunk(p0, 0, 0)                   # Q ec0 chunk 0
        elif g == 1:
            emit_kq_chunk(p0, 0, 1)                   # Q ec0 chunk 1
            for lt in range(0, 8):
                emit_scores(p0, 0, lt, pt_a)
        else:
            for lt in range(4 * g, 4 * g + 4):
                emit_scores(p0, 0, lt, pt_a)

    emit_v_ones(p0)
    for lc in range(NLC):                             # V p0 (dep of attend)
        emit_v_chunk(p0, lc)
    emit_kq_chunk(p0, 0, 2)                           # Q p0 ec1 [filler]
    emit_kq_chunk(p0, 0, 3)
    pt_b = [[None] * NLT, [None] * NLT]
    for lt in range(0, 4):                            # boundary prefill
        emit_scores(p0, 1, lt, pt_b)
    emit_attend(p0, 0, pt_a)

    for lt in range(4, NLT):
        emit_scores(p0, 1, lt, pt_b)
    for lc in range(NLC):                             # K p1 [filler]
        emit_kq_chunk(p1, 1, lc)
    emit_kq_chunk(p1, 0, 0)                           # Q p1 ec0
    emit_kq_chunk(p1, 0, 1)
    pt_a = [[None] * NLT, [None] * NLT]
    for lt in range(0, 4):                            # boundary prefill
        emit_scores(p1, 0, lt, pt_a)
    emit_attend(p0, 1, pt_b)

    for lt in range(4, NLT):
        emit_scores(p1, 0, lt, pt_a)
    emit_v_ones(p1)
    for lc in range(NLC):                             # V p1 [filler + dep]
        emit_v_chunk(p1, lc)
    emit_kq_chunk(p1, 0, 2)                           # Q p1 ec1
    emit_kq_chunk(p1, 0, 3)
    pt_b = [[None] * NLT, [None] * NLT]
    for lt in range(0, 4):                            # boundary prefill
        emit_scores(p1, 1, lt, pt_b)
    emit_attend_sub(p1, 0, 0, pt_a)
    emit_attend_sub(p1, 0, 1, pt_a)

    # block p1ec1: out-proj ec0 groups ride the PE slack of the Act-limited
    # scores stream; tail out-proj follows the norms immediately.
    for lt in range(4, 7):
        emit_scores(p1, 1, lt, pt_b)
    emit_outproj_g(range(0, 2))
    for lt in range(7, 10):
        emit_scores(p1, 1, lt, pt_b)
    emit_outproj_g(range(2, 4))
    for lt in range(10, 13):
        emit_scores(p1, 1, lt, pt_b)
    emit_outproj_g(range(4, 6))
    for lt in range(13, 16):
        emit_scores(p1, 1, lt, pt_b)
    emit_outproj_g(range(6, 8))
    emit_attend_sub(p1, 1, 0, pt_b)
    emit_attend_sub(p1, 1, 1, pt_b)
    emit_outproj_g(range(8, 12), tail=True)
    emit_outproj_g(range(12, 16), tail=True)          # tail


_NC_CACHE = {}


def _get_nc():
    if "nc" not in _NC_CACHE:
        _NC_CACHE["nc"] = build_nc()
    return _NC_CACHE["nc"]


def shard_inputs(x, Wq, bq, Wk, bk, Wv, bv, Wd, bd):
    """Build the 8 per-core input maps, pre-packed into on-chip layouts."""
    x = np.asarray(x, np.float32)
    Wq = np.asarray(Wq, np.float32)
    Wk = np.asarray(Wk, np.float32)
    Wv = np.asarray(Wv, np.float32)
    Wd = np.asarray(Wd, np.float32)
    bq = np.asarray(bq, np.float32)
    bk = np.asarray(bk, np.float32)
    bv = np.asarray(bv, np.float32)

    # x^T per batch in [p, lc, kt, l'] bf16 (the kernel's SBUF layout).
    xts = []
    for b in range(B):
        xT = x[b].T.astype(BF16_NP)                      # [DMODEL, L]
        xts.append(np.ascontiguousarray(
            xT.reshape(KT, P, NLC, LCH).transpose(1, 2, 0, 3)))

    in_maps = []
    for c in range(NCORES):
        b = c // (NCORES // B)
        h0 = (c % (NCORES // B)) * H_PER_CORE
        hs = slice(h0, h0 + H_PER_CORE)
        # [k, pair, i, kt, hd] bf16
        w3 = np.stack(
            [W[:, hs, :].reshape(DMODEL, NPAIR * P)
              .reshape(KT, P, NPAIR, P).transpose(1, 2, 0, 3)
             for W in (Wq, Wk, Wv)], axis=2).astype(BF16_NP)
        # [hd, pair, m] bf16
        wd = (Wd[hs].reshape(NPAIR, P, DMODEL).transpose(1, 0, 2)
              .astype(BF16_NP))
        # [hd, i, pair] f32
        b3 = np.stack([bq[hs].reshape(-1), bk[hs].reshape(-1),
                       bv[hs].reshape(-1)], axis=1).reshape(NPAIR, P, 3)
        b3 = np.ascontiguousarray(b3.transpose(1, 2, 0))
        # [part, pair*P] f32 (bv broadcast across partitions)
        bvr = np.broadcast_to(bv[hs].reshape(1, NPAIR * P), (P, NPAIR * P))
        in_maps.append({
            "xt": xts[b],
            "w": np.ascontiguousarray(w3),
            "wd": np.ascontiguousarray(wd),
            "bias": np.ascontiguousarray(b3),
            "bvr": np.ascontiguousarray(bvr, dtype=np.float32),
        })
    return in_maps


def gather_outputs(results, bd):
    """Sum partial outputs per batch and add bd."""
    out = np.zeros((B, L, DMODEL), np.float32)
    per_b = NCORES // B
    for c, res in enumerate(results):
        out[c // per_b] += np.asarray(res["y"], np.float32)
    out += np.asarray(bd, np.float32)[None, None, :]
    return out


def kernel(x, Wq, bq, Wk, bk, Wv, bv, Wd, bd, _trace=False):
    nc = _get_nc()
    in_maps = shard_inputs(x, Wq, bq, Wk, bk, Wv, bv, Wd, bd)
    res = run_bass_kernel_spmd(nc, in_maps, list(range(NCORES)), trace=_trace)
    out = gather_outputs(res.results, bd)
    if _trace:
        kernel.last_results = res
    return out


# revision 51
# speedup vs baseline: 1.1901x; 1.0095x over previous
"""Trainium2 Bass kernel: multi-head attention (dense transformer block).

Computation (per batch b):
    Q = x @ Wq + bq ; K = x @ Wk + bk ; V = x @ Wv + bv        (per head)
    P = exp((Q @ K^T) / sqrt(Dh))                   (no max-subtraction needed:
                                                     scores are O(1) by construction)
    out = sum_h (P @ V / rowsum(P)) @ Wd[h] + bd

Sharding (data + tensor parallel): 8 cores; core c handles batch b = c // 4
and the 4 heads starting at 4*(c % 4). Each core computes a partial [L, D]
output; the host sums the 4 partials per batch and adds bd.

Host-side layout prep: x is pre-transposed to x^T and pre-cast to bf16 (the
same rounding the kernel used to do on-chip), weights are pre-packed into
their exact on-chip bf16 layouts. All DMAs are contiguous HWDGE transfers,
and the on-chip x-transpose/cast pipeline (which dominated the old lead-in)
disappears entirely.

Schedule: hand-interleaved emission so the PE never idles and the Act engine
runs exp (its ~128us is the #2 engine floor) nearly continuously:
  - K/Q chunks follow each x^T l-chunk DMA; scores stream right behind.
  - pair-1 QKV, V-proj and the ec0 out-projection are emitted inside other
    blocks' exp-lag windows as PE filler.
  - Act engine does ONLY exp; drains/biases live on DVE & Pool.
  - V computed as V^T (weight-stationary J=512, LDWEIGHTS hidden) then
    PE-transposed back; ones-columns give the softmax denominator for free.
  - normalize: reciprocal_approx_fast (DVE) + Pool multiply.
  - out-proj per ec with per-tile y DMA, overlapped with ec1 compute.
"""

import os
import sys
from contextlib import ExitStack

import numpy as np

try:
    from ml_dtypes import bfloat16 as _BF16_NP
except ImportError:                      # fall back to jax's bundled dtype
    from jax.numpy import bfloat16 as _BF16_NP

for _p in ("/opt/trn_rl_repo", "/root/.axon_site/_ro/trn_rl_repo"):
    if os.path.isdir(_p) and _p not in sys.path:
        sys.path.append(_p)

import concourse.bass as bass
import concourse.tile as tile
from concourse import bacc, mybir
from concourse.bass import ds, ts
from concourse.bass_utils import run_bass_kernel_spmd
from concourse.masks import make_identity

F32 = mybir.dt.float32
BF16 = mybir.dt.bfloat16
BF16_NP = _BF16_NP

# Problem sizes (hardcoded per contract).
DMODEL, HEADS, DHEAD = 1024, 16, 64
B, L = 2, 2048
NCORES = 8
H_PER_CORE = B * HEADS // NCORES          # 4 heads per core
NPAIR = H_PER_CORE // 2                   # head pairs per core (= 2)
P = 128                                   # partitions
KT = DMODEL // P                          # 8 k-tiles over dmodel
NLT = L // P                              # 16 l-tiles
LCH = 512                                 # matmul free-dim chunk (one psum bank)
NLC = L // LCH                            # 4 l-chunks
ECH = 1024                                # exp chunk (2 psum banks)
NEC = L // ECH                            # 2 exp chunks
MCH = 512                                 # m-chunk for out-proj
NMC = DMODEL // MCH


def build_nc():
    """Build the SPMD Bass program for one core."""
    nc = bacc.Bacc("TRN2", target_bir_lowering=False, debug=False,
                   num_devices=NCORES)

    # Inputs are pre-packed host-side into on-chip layouts (see shard_inputs).
    xt_d = nc.dram_tensor("xt", [P, NLC, KT, LCH], BF16, kind="ExternalInput").ap()
    w_d = nc.dram_tensor("w", [P, NPAIR, 3, KT, P], BF16, kind="ExternalInput").ap()
    wd_d = nc.dram_tensor("wd", [P, NPAIR, DMODEL], BF16, kind="ExternalInput").ap()
    bias_d = nc.dram_tensor("bias", [P, 3, NPAIR], F32, kind="ExternalInput").ap()
    bvr_d = nc.dram_tensor("bvr", [P, NPAIR * P], F32, kind="ExternalInput").ap()
    y_d = nc.dram_tensor("y", [L, DMODEL], BF16, kind="ExternalOutput").ap()

    with ExitStack() as ctx:
        tc = ctx.enter_context(tile.TileContext(nc))
        _body(nc, tc, ctx, xt_d, w_d, wd_d, bias_d, bvr_d, y_d)
    nc.compile()
    return nc


def _body(nc, tc, ctx, xt_d, w_d, wd_d, bias_d, bvr_d, y_d):
    const = ctx.enter_context(tc.tile_pool(name="const", bufs=1))
    sb = ctx.enter_context(tc.tile_pool(name="sb", bufs=1))
    psum = ctx.enter_context(tc.tile_pool(name="psum", bufs=1, space="PSUM"))

    ident = const.tile([P, P], BF16)
    make_identity(nc, ident)

    # ---- persistent SBUF tensors (DMA'd directly, no staging) ----
    xt = sb.tile([P, NLC, KT, LCH], BF16)                # x^T, bf16
    w_sb = const.tile([P, NPAIR, 3, KT, P], BF16)        # Wq/Wk/Wv per pair
    wd_sb = const.tile([P, NPAIR, DMODEL], BF16)
    bias_sb = const.tile([P, 3, NPAIR], F32)
    bv_rep = const.tile([P, NPAIR * P], F32)
    kT = [sb.tile([P, L], BF16, tag="kT", bufs=NPAIR, name=f"kT{p}")
          for p in range(NPAIR)]
    qT = [sb.tile([P, L], BF16, tag="qT", bufs=NPAIR, name=f"qT{p}")
          for p in range(NPAIR)]
    vt = [sb.tile([P, NLT, 2 * P], BF16, tag="vt", bufs=NPAIR, name=f"vt{p}")
          for p in range(NPAIR)]
    o_norm = sb.tile([P, NPAIR, L], BF16)

    # ---- emission helpers (order of calls == engine queue order) ----

    def emit_kq_chunk(p, i, lc):
        """One 512-wide chunk of K^T (i=1) or Q^T (i=0) for pair p."""
        dst = qT[p] if i == 0 else kT[p]
        ps = psum.tile([P, LCH], F32, tag="qkvp", bufs=2, name="qkvps")
        for kt in range(KT):
            nc.tensor.matmul(ps, lhsT=w_sb[:, p, i, kt],
                             rhs=xt[:, lc, kt],
                             start=(kt == 0), stop=(kt == KT - 1))
        nc.vector.tensor_scalar_add(dst[:, ds(lc * LCH, LCH)], ps,
                                    bias_sb[:, i, p:p + 1])

    def emit_v_ones(p):
        """memset the ones-columns of vt[p] (softmax denominator trick)."""
        v4 = vt[p].rearrange("q l (b c) -> q l b c", b=2)
        nc.gpsimd.memset(v4[:, :, :, DHEAD:P], 1.0)

    def emit_v_chunk(p, lc):
        """V for l-tiles 4lc..4lc+3 of pair p: V^T with weight-stationary
        J=512 matmuls (LDWEIGHTS hidden), PE-transpose back to [l', d],
        bias added on the strided drain."""
        vps = psum.tile([P, LCH], F32, tag="qkvp", bufs=2, name="vps")
        for kt in range(KT):
            nc.tensor.matmul(vps, lhsT=w_sb[:, p, 2, kt],
                             rhs=xt[:, lc, kt],
                             start=(kt == 0), stop=(kt == KT - 1))
        vts = sb.tile([P, LCH], BF16, tag="vts", bufs=1)
        nc.vector.tensor_copy(vts, vps)
        tp = psum.tile([P, 4, P], BF16, tag="op", bufs=2, name="vtp")
        for j in range(4):
            nc.tensor.transpose(tp[:, j], vts[:, ds(j * P, P)], ident)
        bvs = bv_rep[:, ds(p * P, P)].rearrange("q (b c) -> q b c", b=2)
        for j in range(4):
            lt = 4 * lc + j
            dst = vt[p].rearrange("q l (b c) -> q l b c", b=2)[:, lt, :, 0:DHEAD]
            nc.vector.tensor_add(dst, tp[:, j].rearrange("q (b c) -> q b c", b=2), bvs)

    def emit_scores(p, ec, lt, pt_tiles):
        """Scores + exp for one key l-tile of (pair, ec): 2 heads dual-tile."""
        for h in range(2):
            sp = psum.tile([P, ECH], F32, tag="sc", bufs=2, name="sp")
            for sub in range(ECH // LCH):
                nc.tensor.matmul(
                    sp[:, ds(sub * LCH, LCH)],
                    lhsT=kT[p][ds(64 * h, 64), ds(lt * P, P)],
                    rhs=qT[p][ds(64 * h, 64), ds(ec * ECH + sub * LCH, LCH)],
                    start=True, stop=True)
            pt = sb.tile([P, ECH], BF16, tag="pt", bufs=40)
            nc.scalar.activation(pt, sp, func=mybir.ActivationFunctionType.Exp,
                                 scale=1.0 / np.sqrt(DHEAD))
            pt_tiles[h][lt] = pt

    def emit_attend_sub(p, ec, sub, pt_tiles, filler=()):
        """One 512-query sub-chunk: h0+h1 chains interleaved so both track
        the exp stream; then normalize both heads. `filler` thunks are
        drizzled between chain steps (PE slack)."""
        filler = list(filler)
        lc = ec * ECH + sub * LCH
        ops = [psum.tile([P, LCH], F32, tag="op", bufs=2, name=f"oph{h}")
               for h in range(2)]
        for lt in range(NLT):
            for h in range(2):
                nc.tensor.matmul(
                    ops[h], lhsT=vt[p][:, lt, ds(P * h, P)],
                    rhs=pt_tiles[h][lt][:, ds(sub * LCH, LCH)],
                    start=(lt == 0), stop=(lt == NLT - 1))
            if lt % 2 == 1 and filler:
                filler.pop(0)()
        for h in range(2):
            op = ops[h]
            # rows 64..127 hold the softmax denominator (ones columns).
            # Builtin copies handle the partition shift 64->0; the custom
            # approx-reciprocal then runs partition-aligned.
            os_sb = sb.tile([DHEAD, LCH], F32, tag="os", bufs=2)
            nc.vector.tensor_copy(os_sb, op[0:DHEAD, :])
            den = sb.tile([DHEAD, LCH], F32, tag="den", bufs=2)
            nc.vector.tensor_copy(den, op[DHEAD:P, :])
            rs = sb.tile([DHEAD, LCH], F32, tag="rs", bufs=2)
            nc.vector.reciprocal_approx_fast(rs, den)
            nc.gpsimd.tensor_mul(
                o_norm[ds(64 * h, 64), p, ds(lc, LCH)], os_sb, rs)

    def emit_attend(p, ec, pt_tiles):
        for sub in range(ECH // LCH):
            emit_attend_sub(p, ec, sub, pt_tiles)

    _opj = [0]

    def emit_outproj_unit(lt, mc, tail=False):
        """One [128, 512] Y tile. In the tail, rotate psum tags (all free
        by then) to deepen the pipeline, split drains Act/DVE, and fan the
        y DMAs across three queues."""
        k = _opj[0]
        _opj[0] += 1
        tag = "qkvp"
        yp = psum.tile([P, MCH], F32, tag=tag, bufs=2, name=f"yp_{tag}")
        for p in range(NPAIR):
            nc.tensor.matmul(
                yp, lhsT=o_norm[:, p, ds(lt * P, P)],
                rhs=wd_sb[:, p, ds(mc * MCH, MCH)],
                start=(p == 0), stop=(p == NPAIR - 1))
        ys = sb.tile([P, MCH], BF16, tag="ys", bufs=3)
        if tail:
            nc.scalar.copy(ys, yp)
        else:
            nc.vector.tensor_copy(ys, yp)
        dst = y_d[ds(lt * P, P), ds(mc * MCH, MCH)]
        if tail and k % 3 == 1:
            nc.scalar.dma_start(dst, ys)
        elif tail and k % 3 == 2:
            nc.gpsimd.dma_start(dst, ys)
        else:
            nc.sync.dma_start(dst, ys)

    def emit_outproj_g(lts, tail=False):
        for lt in lts:
            for mc in range(NMC):
                emit_outproj_unit(lt, mc, tail)

    # ================= global emission order =================
    pt_a = [[None] * NLT, [None] * NLT]   # pt tiles for the active block
    p0, p1 = 0, 1

    # Input DMAs: K/Q weights of pair 0 first (they gate the first chunks),
    # then x^T chunks; everything else rides the gpsimd queue in parallel.
    nc.scalar.dma_start(w_sb[:, p0, 1], w_d[:, p0, 1])
    nc.scalar.dma_start(w_sb[:, p0, 0], w_d[:, p0, 0])
    nc.gpsimd.dma_start(bias_sb, bias_d)
    nc.gpsimd.dma_start(w_sb[:, p0, 2], w_d[:, p0, 2])
    nc.gpsimd.dma_start(bv_rep, bvr_d)
    for kq in range(4):                               # chunk 0 in quarters
        nc.sync.dma_start(xt[:, 0, ds(2 * kq, 2)], xt_d[:, 0, ds(2 * kq, 2)])
    for lc in range(1, NLC):
        nc.sync.dma_start(xt[:, lc, 0:KT // 2], xt_d[:, lc, 0:KT // 2])
        nc.sync.dma_start(xt[:, lc, KT // 2:], xt_d[:, lc, KT // 2:])
    nc.gpsimd.dma_start(w_sb[:, p1], w_d[:, p1])
    nc.gpsimd.dma_start(wd_sb, wd_d)

    # K/Q(ec0) p0 + scores(p0, ec0), streaming behind the x^T chunk DMAs.
    # scores(lt) needs K chunk lt//4 and BOTH Q chunks of ec0.
    for g in range(4):
        emit_kq_chunk(p0, 1, g)                       # K chunk g
        if g == 0:
            emit_kq_chunk(p0, 0, 0)                   # Q ec0 chunk 0
        elif g == 1:
            emit_kq_chunk(p0, 0, 1)                   # Q ec0 chunk 1
            for lt in range(0, 8):
                emit_scores(p0, 0, lt, pt_a)
        else:
            for lt in range(4 * g, 4 * g + 4):
                emit_scores(p0, 0, lt, pt_a)

    emit_v_ones(p0)
    for lc in range(NLC):                             # V p0 (dep of attend)
        emit_v_chunk(p0, lc)
    emit_kq_chunk(p0, 0, 2)                           # Q p0 ec1 [filler]
    emit_kq_chunk(p0, 0, 3)
    pt_b = [[None] * NLT, [None] * NLT]
    for lt in range(0, 4):                            # boundary prefill
        emit_scores(p0, 1, lt, pt_b)
    emit_attend(p0, 0, pt_a)

    for lt in range(4, NLT):
        emit_scores(p0, 1, lt, pt_b)
    for lc in range(NLC):                             # K p1 [filler]
        emit_kq_chunk(p1, 1, lc)
    emit_kq_chunk(p1, 0, 0)                           # Q p1 ec0
    emit_kq_chunk(p1, 0, 1)
    pt_a = [[None] * NLT, [None] * NLT]
    for lt in range(0, 4):                            # boundary prefill
        emit_scores(p1, 0, lt, pt_a)
    emit_attend(p0, 1, pt_b)

    for lt in range(4, NLT):
        emit_scores(p1, 0, lt, pt_a)
    emit_v_ones(p1)
    for lc in range(NLC):                             # V p1 [filler + dep]
        emit_v_chunk(p1, lc)
    emit_kq_chunk(p1, 0, 2)                           # Q p1 ec1
    emit_kq_chunk(p1, 0, 3)
    pt_b = [[None] * NLT, [None] * NLT]
    for lt in range(0, 4):                            # boundary prefill
        emit_scores(p1, 1, lt, pt_b)
    emit_attend_sub(p1, 0, 0, pt_a)
    emit_attend_sub(p1, 0, 1, pt_a)

    # block p1ec1: out-proj ec0 groups ride the PE slack of the Act-limited
    # scores stream; tail out-proj follows the norms immediately.
    for lt in range(4, 7):
        emit_scores(p1, 1, lt, pt_b)
    emit_outproj_g(range(0, 2))
    for lt in range(7, 10):
        emit_scores(p1, 1, lt, pt_b)
    emit_outproj_g(range(2, 4))
    for lt in range(10, 13):
        emit_scores(p1, 1, lt, pt_b)
    emit_outproj_g(range(4, 6))
    for lt in range(13, 16):
        emit_scores(p1, 1, lt, pt_b)
    emit_outproj_g(range(6, 8))
    emit_attend_sub(p1, 1, 0, pt_b)
    emit_attend_sub(p1, 1, 1, pt_b)
    emit_outproj_g(range(8, 12), tail=True)
    emit_outproj_g(range(12, 16), tail=True)          # tail


_NC_CACHE = {}


def _get_nc():
    if "nc" not in _NC_CACHE:
        _NC_CACHE["nc"] = build_nc()
    return _NC_CACHE["nc"]


def shard_inputs(x, Wq, bq, Wk, bk, Wv, bv, Wd, bd):
    """Build the 8 per-core input maps, pre-packed into on-chip layouts."""
    x = np.asarray(x, np.float32)
    Wq = np.asarray(Wq, np.float32)
    Wk = np.asarray(Wk, np.float32)
    Wv = np.asarray(Wv, np.float32)
    Wd = np.asarray(Wd, np.float32)
    bq = np.asarray(bq, np.float32)
    bk = np.asarray(bk, np.float32)
    bv = np.asarray(bv, np.float32)

    # x^T per batch in [p, lc, kt, l'] bf16 (the kernel's SBUF layout).
    xts = []
    for b in range(B):
        xT = x[b].T.astype(BF16_NP)                      # [DMODEL, L]
        xts.append(np.ascontiguousarray(
            xT.reshape(KT, P, NLC, LCH).transpose(1, 2, 0, 3)))

    in_maps = []
    for c in range(NCORES):
        b = c // (NCORES // B)
        h0 = (c % (NCORES // B)) * H_PER_CORE
        hs = slice(h0, h0 + H_PER_CORE)
        # [k, pair, i, kt, hd] bf16
        w3 = np.stack(
            [W[:, hs, :].reshape(DMODEL, NPAIR * P)
              .reshape(KT, P, NPAIR, P).transpose(1, 2, 0, 3)
             for W in (Wq, Wk, Wv)], axis=2).astype(BF16_NP)
        # [hd, pair, m] bf16
        wd = (Wd[hs].reshape(NPAIR, P, DMODEL).transpose(1, 0, 2)
              .astype(BF16_NP))
        # [hd, i, pair] f32
        b3 = np.stack([bq[hs].reshape(-1), bk[hs].reshape(-1),
                       bv[hs].reshape(-1)], axis=1).reshape(NPAIR, P, 3)
        b3 = np.ascontiguousarray(b3.transpose(1, 2, 0))
        # [part, pair*P] f32 (bv broadcast across partitions)
        bvr = np.broadcast_to(bv[hs].reshape(1, NPAIR * P), (P, NPAIR * P))
        in_maps.append({
            "xt": xts[b],
            "w": np.ascontiguousarray(w3),
            "wd": np.ascontiguousarray(wd),
            "bias": np.ascontiguousarray(b3),
            "bvr": np.ascontiguousarray(bvr, dtype=np.float32),
        })
    return in_maps


def gather_outputs(results, bd):
    """Sum partial outputs per batch and add bd."""
    out = np.zeros((B, L, DMODEL), np.float32)
    per_b = NCORES // B
    for c, res in enumerate(results):
        out[c // per_b] += np.asarray(res["y"], np.float32)
    out += np.asarray(bd, np.float32)[None, None, :]
    return out


def kernel(x, Wq, bq, Wk, bk, Wv, bv, Wd, bd, _trace=False):
    nc = _get_nc()
    in_maps = shard_inputs(x, Wq, bq, Wk, bk, Wv, bv, Wd, bd)
    res = run_bass_kernel_spmd(nc, in_maps, list(range(NCORES)), trace=_trace)
    out = gather_outputs(res.results, bd)
    if _trace:
        kernel.last_results = res
    return out
